# revision 11
# baseline (speedup 1.0000x reference)
"""Trainium2 Bass kernel: DiscreteEmbedding (rect-window embedding lookup).

Math (matches the jax reference):
    xs  = x * 2048;  y = xs + 0.5
    i_lo = ceil(y)-1  (robust fp32 compare fixup for the HW int round mode)
    plain tokens:    out = T[i_lo]           (T[2048] := 0 for the xs>2047.5 tail)
    boundary tokens: out = 0.5*(T[i_lo] + T[i_lo+1])   (y exactly integer)

Device strategy (8 cores, data-parallel over tokens; 8192 tokens/core):
  - ONE bf16 gather per token from a combined DRAM table TC[4224, 128]:
      rows 0..2047  = bf16(T)
      rows 2048..2175 = 0      (row 2048 serves the xs>2047.5 tail)
      rows 2176..4223 = avg[k] = 0.5*(T[k]+T[k+1])  (T[2048]=0)
    idx2 = i_lo + 2176*boundary.  bf16 halves gather + store HBM traffic
    (rel err ~1.7e-3 from table rounding; gate is 2e-2).
  - dma_gather on 4 SWDGE queues with prepare_only=True: Q7 descriptor
    generation (~8ns/idx/queue pair, the critical path) starts as soon as
    idx16 is ready, concurrent with the whole table build.  trigger_dma
    fires each chunk only after the TC stores complete (tblsem), which is
    the DRAM RAW guard.  Output stores wait each chunk's DMA sem.
  - Warmup gathers are the FIRST Pool ops so the Q7 dma_gather IRAM load
    (~6us) overlaps the DMA preamble + loads; no gpsimd memsets anywhere
    (avoids Q7 module thrash).
  - Gather output is position-ordered [128, j, 128]; stores are contiguous;
    host un-permutes rows and upcasts bf16->fp32 (layout/dtype only).
"""

import numpy as np
import ml_dtypes

import concourse.mybir as mybir
import concourse.tile as tile
from concourse import bacc, bass_utils

N_CORES = 8
B, S = 32, 2048
V, D = 2048, 128
TOK = B * S                 # 65536 tokens total
TPC = TOK // N_CORES        # 8192 tokens per core
SPC = TPC // 16             # 512: free dim of the wrapped [16, 512] x layout
ABASE = 2176                # avg rows start
VEXT = 4224                 # TC rows
NQ = 4                      # SWDGE queues
CHUNK = 1024                # gather idx per chunk (2 chunks per queue)
NCHUNK = TPC // CHUNK       # 8

F32 = mybir.dt.float32
I32 = mybir.dt.int32
I16 = mybir.dt.int16
BF16 = mybir.dt.bfloat16
OP = mybir.AluOpType


def build():
    nc = bacc.Bacc(
        "TRN2",
        target_bir_lowering=False,
        debug=False,
        num_devices=N_CORES,
        num_swdge_queues=NQ,
    )
    xr = nc.dram_tensor("xr", [128, SPC], F32, kind="ExternalInput")
    emb = nc.dram_tensor("emb", [V, D], F32, kind="ExternalInput")
    out = nc.dram_tensor("out", [TPC, D], BF16, kind="ExternalOutput")
    tcd = nc.dram_tensor("tcd", [VEXT, D], BF16, kind="Internal")

    with tile.TileContext(nc) as tc:
        with tc.tile_pool(name="sb", bufs=1) as sb, tc.tile_pool(name="g", bufs=1) as gp:
            # ---- warmups first: Q7 dma_gather IRAM load + per-queue ring
            # init overlap the preamble/loads instead of gating desc-gen.
            zidx = sb.tile([128, 1], I16)
            nc.vector.memset(zidx[:], 0)
            for q in range(NQ):
                wg = sb.tile([128, D], F32, tag=f"warm{q}")
                nc.gpsimd.dma_gather(
                    wg[:].rearrange("p (j d) -> p j d", d=D),
                    emb[:],
                    zidx[:, 0:1],
                    num_idxs=16,
                    num_idxs_reg=16,
                    elem_size=D,
                    single_packet=False,
                    queue_num=q,
                )

            # ---- loads: x on the SP ring, table on the ACT ring ----
            xt = sb.tile([128, SPC], F32)
            nc.sync.dma_start(out=xt[:], in_=xr[:])
            tbl32 = sb.tile([128, (V // 128) * D], F32)
            nc.scalar.dma_start(
                out=tbl32[:].rearrange("p (r d) -> p r d", d=D),
                in_=emb[:].rearrange("(r p) d -> p r d", p=128),
            )

            # ---- index math (fp32, exact): y = x*2048 + 0.5 ----
            y = sb.tile([128, SPC], F32)
            nc.vector.tensor_scalar(y[:], xt[:], 2048.0, 0.5, op0=OP.mult, op1=OP.add)
            i0 = sb.tile([128, SPC], I32)
            nc.vector.tensor_copy(i0[:], y[:])
            f0 = sb.tile([128, SPC], F32)
            nc.vector.tensor_copy(f0[:], i0[:])
            lt = sb.tile([128, SPC], F32)    # f0 < y
            nc.vector.tensor_tensor(lt[:], f0[:], y[:], op=OP.is_lt)
            bnd = sb.tile([128, SPC], F32)   # y integer -> avg row
            nc.vector.tensor_tensor(bnd[:], f0[:], y[:], op=OP.is_equal)
            lf = sb.tile([128, SPC], F32)    # i_lo = ceil(y) - 1
            nc.vector.tensor_add(lf[:], f0[:], lt[:])
            nc.vector.tensor_scalar_add(lf[:], lf[:], -1.0)
            idxf = sb.tile([128, SPC], F32)  # idx2 = i_lo + 2176*bnd
            nc.vector.scalar_tensor_tensor(
                out=idxf[:], in0=bnd[:], scalar=float(ABASE), in1=lf[:],
                op0=OP.mult, op1=OP.add,
            )
            idx16 = sb.tile([128, SPC], I16)
            nc.vector.tensor_copy(idx16[:], idxf[:])

            # ---- combined table (bf16, SBUF) -> DRAM tcd ----
            tcbf = sb.tile([128, 17 * D], BF16)   # plain ranks + zero rank
            nc.vector.memset(tcbf[:, 16 * D : 17 * D], 0.0)
            nc.vector.tensor_copy(tcbf[:, 0 : 16 * D], tbl32[:])
            shiftbf = sb.tile([128, 16 * D], BF16)  # T[r*128+p+1]
            nc.sync.dma_start(out=shiftbf[0:127, :], in_=tcbf[1:128, 0 : 16 * D])
            nc.sync.dma_start(
                out=shiftbf[127:128, :], in_=tcbf[0:1, D : 17 * D]
            )
            avgbf = sb.tile([128, 16 * D], BF16)
            nc.vector.tensor_add(avgbf[:], tcbf[:, 0 : 16 * D], shiftbf[:])
            nc.vector.tensor_scalar_mul(avgbf[:], avgbf[:], 0.5)

            st_p = nc.scalar.dma_start(
                out=tcd[0:V].rearrange("(r p) d -> p r d", p=128),
                in_=tcbf[:, 0 : 16 * D].rearrange("p (r d) -> p r d", d=D),
            )
            st_z = nc.sync.dma_start(
                out=tcd[V : ABASE].rearrange("(r p) d -> p r d", p=128),
                in_=tcbf[:, 16 * D : 17 * D].rearrange("p (r d) -> p r d", d=D),
            )
            st_a = nc.sync.dma_start(
                out=tcd[ABASE:VEXT].rearrange("(r p) d -> p r d", p=128),
                in_=avgbf[:].rearrange("p (r d) -> p r d", d=D),
            )
            tc_stores = [st_p, st_z, st_a]

            # ---- chunked gathers (triggered), round-robin over queues ----
            from concourse.tile import add_dep_helper

            nreg = nc.gpsimd.to_reg(CHUNK)
            out_v = out[:].rearrange("(p j) d -> p (j d)", p=128)
            JB = CHUNK // 128                 # j-blocks per chunk
            for ci in range(NCHUNK):
                q = ci % NQ
                g = gp.tile([128, JB * D], BF16, tag=f"g{ci}")
                gi = nc.gpsimd.dma_gather(
                    g[:].rearrange("p (j d) -> p j d", d=D),
                    tcd[:],
                    idx16[:, ci * (CHUNK // 16) : (ci + 1) * (CHUNK // 16)],
                    num_idxs=CHUNK,
                    num_idxs_reg=nreg,
                    elem_size=D,
                    single_packet=False,
                    queue_num=q,
                )
                if ci == 0:
                    # RAW guard: gather SDMA reads tcd; Tile does not thread
                    # DRAM deps.  Pool program order extends to later chunks.
                    for st in tc_stores:
                        add_dep_helper(gi.ins, st.ins, True, "tcd RAW guard")
                eng = nc.sync if ci % 2 == 0 else nc.scalar
                eng.dma_start(
                    out=out_v[:, ci * JB * D : (ci + 1) * JB * D], in_=g[:]
                )
    nc.compile()
    return nc


_NC = None


def _row_perm():
    """out row r holds gather position i(r) = (r%64)*128 + r//64; position i
    handles token t(i) = (i%16)*512 + i//16 (x wrapped [16,512])."""
    r = np.arange(TPC)
    p, j = r // 64, r % 64
    i = j * 128 + p
    return (i % 16) * SPC + i // 16


def kernel(x, time_embedding):
    global _NC
    x = np.ascontiguousarray(np.asarray(x, dtype=np.float32))
    t = np.ascontiguousarray(np.asarray(time_embedding, dtype=np.float32))
    xf = x.reshape(-1)
    in_maps = []
    for c in range(N_CORES):
        xc = xf[c * TPC : (c + 1) * TPC].reshape(16, SPC)
        in_maps.append({"xr": np.ascontiguousarray(np.tile(xc, (8, 1))), "emb": t})

    if _NC is None:
        _NC = build()
    res = bass_utils.run_bass_kernel_spmd(_NC, in_maps, core_ids=list(range(N_CORES)))
    global _LAST_RES
    _LAST_RES = res

    tkn = _row_perm()
    outs = []
    for c in range(N_CORES):
        oc = np.asarray(res.results[c]["out"]).astype(np.float32)
        full = np.empty_like(oc)
        full[tkn] = oc
        outs.append(full)
    return np.concatenate(outs, axis=0).reshape(B, S, D)


# revision 13
# speedup vs baseline: 1.3987x; 1.3987x over previous
"""Trainium2 Bass kernel: DiscreteEmbedding (rect-window embedding lookup).

Math (matches the jax reference):
    xs  = x * 2048;  y = xs + 0.5
    i_lo = ceil(y)-1  (robust fp32 compare fixup for the HW int round mode)
    plain tokens:    out = T[i_lo]           (T[2048] := 0 for the xs>2047.5 tail)
    boundary tokens: out = 0.5*(T[i_lo] + T[i_lo+1])   (y exactly integer)

Device strategy (8 cores, data-parallel over tokens; 8192 tokens/core):
  - ONE bf16 gather per token from an INTERLEAVED pair table in DRAM:
      tcd[2v]   = bf16(T[v])          v = 0..2047
      tcd[2v+1] = bf16(0.5*(T[v]+T[v+1]))   (T[2048]=0)
      tcd[4096] = 0                   (the xs>2047.5 tail)
    so idx' = 2*i_lo + boundary -- two DVE ops, no floor/mod remap.
  - Block-major SBUF layout (partition p holds rows 16p..16p+15): the avg
    operand T[w+1] is an intra-partition shifted view; only the 16 rows
    crossing partitions need the tiny strided `tnext` load.  Every big DMA
    (x, table load, table store, output stores) is contiguous per
    partition -- no descriptor storms on the rings.
  - Gathers run triggered on 4 SWDGE queues (desc-gen ~8ns/idx/queue is
    the critical path).  Warmup gathers are the first Pool ops so the Q7
    IRAM load (~10us) overlaps the preamble + table build; trailing table
    dependency is only the contiguous 1MB store.
  - ACT computes th=0.5*T while DVE interleaves idx math with the bf16
    cast/avg so both finish inside the IRAM window.
  - Host: un-permute gather positions to token order, upcast bf16->fp32
    (layout/dtype only; every value is HW-produced).
"""

import numpy as np
import ml_dtypes

import concourse.mybir as mybir
import concourse.tile as tile
from concourse.tile import add_dep_helper
from concourse import bacc, bass_utils

N_CORES = 8
B, S = 32, 2048
V, D = 2048, 128
TOK = B * S
TPC = TOK // N_CORES        # 8192 tokens per core
SPC = TPC // 16             # 512
VEXT = 4224                 # pair table rows (4096 pairs + zero row + pad)
NQ = 4
CHUNK = 1024
NCHUNK = TPC // CHUNK       # 8
JB = CHUNK // 128           # 8 j-blocks per chunk

F32 = mybir.dt.float32
I32 = mybir.dt.int32
I16 = mybir.dt.int16
BF16 = mybir.dt.bfloat16
OP = mybir.AluOpType
AF = mybir.ActivationFunctionType


def build():
    nc = bacc.Bacc(
        "TRN2",
        target_bir_lowering=False,
        debug=False,
        num_devices=N_CORES,
        num_swdge_queues=NQ,
    )
    xr = nc.dram_tensor("xr", [128, SPC], F32, kind="ExternalInput")
    emb = nc.dram_tensor("emb", [V, D], F32, kind="ExternalInput")
    out = nc.dram_tensor("out", [128, (TPC // 128) * D], BF16, kind="ExternalOutput")
    tcd = nc.dram_tensor("tcd", [VEXT, D], BF16, kind="Internal")

    with tile.TileContext(nc) as tc:
        with tc.tile_pool(name="sb", bufs=1) as sb, tc.tile_pool(name="g", bufs=1) as gp:
            # ---- warmups ASAP: Q7 dma_gather IRAM load (~10us) + ring init
            # overlap the preamble/loads.  Keep Pool pre-ops minimal.
            zidx = sb.tile([128, 1], I16)
            nc.vector.memset(zidx[:], 0)
            wreg = nc.gpsimd.to_reg(16)
            for q in range(NQ):
                wg = sb.tile([128, D], F32, tag=f"warm{q}")
                nc.gpsimd.dma_gather(
                    wg[:].rearrange("p (j d) -> p j d", d=D),
                    emb[:],
                    zidx[:, 0:1],
                    num_idxs=16,
                    num_idxs_reg=wreg,
                    elem_size=D,
                    single_packet=False,
                    queue_num=q,
                )

            # ---- loads: x halves on both rings first, then the table ----
            xt = sb.tile([128, SPC], F32)
            nc.sync.dma_start(out=xt[0:64, :], in_=xr[0:64, :])
            nc.scalar.dma_start(out=xt[64:128, :], in_=xr[64:128, :])
            tbl32 = sb.tile([128, 16 * D], F32)   # row 16p+n at (p, n)
            nc.scalar.dma_start(
                out=tbl32[:], in_=emb[:].rearrange("(p n) d -> p (n d)", p=128)
            )
            tnext = sb.tile([128, D], F32)        # T[16(p+1)], T[2048]=0
            nc.vector.memset(tnext[:], 0.0)
            nc.sync.dma_start(
                out=tnext[0:127, :],
                in_=emb[16:V].rearrange("(p n) d -> p (n d)", p=127)[:, 0:D],
            )
            zrow = sb.tile([1, D], BF16)
            nc.vector.memset(zrow[:], 0.0)

            # th = 0.5*T on ACT (the avg operand); thn = 0.5*tnext
            th = sb.tile([128, 16 * D], F32)
            nc.scalar.activation(th[:], tbl32[:], AF.Copy, scale=0.5)
            thn = sb.tile([128, D], F32)
            nc.scalar.activation(thn[:], tnext[:], AF.Copy, scale=0.5)

            # ---- DVE: idx compares first, then table ops (while ACT runs),
            # then the rest of the idx chain.
            y = sb.tile([128, SPC], F32)
            nc.vector.tensor_scalar(y[:], xt[:], 2048.0, 0.5, op0=OP.mult, op1=OP.add)
            i0 = sb.tile([128, SPC], I32)
            nc.vector.tensor_copy(i0[:], y[:])
            f0 = sb.tile([128, SPC], F32)
            nc.vector.tensor_copy(f0[:], i0[:])
            lt = sb.tile([128, SPC], F32)
            nc.vector.tensor_tensor(lt[:], f0[:], y[:], op=OP.is_lt)
            bnd = sb.tile([128, SPC], F32)
            nc.vector.tensor_tensor(bnd[:], f0[:], y[:], op=OP.is_equal)

            # interleaved pair table in SBUF: slot 2n = T[16p+n], 2n+1 = avg
            tcbf = sb.tile([128, 32 * D], BF16)
            t4 = tcbf[:].rearrange("p (n t d) -> p n t d", t=2, d=D)
            nc.vector.tensor_copy(
                t4[:, :, 0:1, :], tbl32[:].rearrange("p (n u d) -> p n u d", u=1, d=D)
            )
            nc.vector.tensor_tensor(
                t4[:, 0:15, 1:2, :],
                th[:, 0 : 15 * D].rearrange("p (n u d) -> p n u d", u=1, d=D),
                th[:, D : 16 * D].rearrange("p (n u d) -> p n u d", u=1, d=D),
                op=OP.add,
            )
            nc.vector.tensor_tensor(
                t4[:, 15:16, 1:2, :],
                th[:, 15 * D : 16 * D].rearrange("p (n u d) -> p n u d", u=1, d=D),
                thn[:].rearrange("p (n u d) -> p n u d", u=1, d=D),
                op=OP.add,
            )

            # rest of idx chain: idx' = 2*(f0 + lt - 1) + bnd
            lf = sb.tile([128, SPC], F32)
            nc.vector.scalar_tensor_tensor(
                out=lf[:], in0=lt[:], scalar=-1.0, in1=f0[:], op0=OP.add, op1=OP.add
            )
            idxf = sb.tile([128, SPC], F32)
            nc.vector.scalar_tensor_tensor(
                out=idxf[:], in0=lf[:], scalar=2.0, in1=bnd[:], op0=OP.mult, op1=OP.add
            )
            idx16 = sb.tile([128, SPC], I16)
            nc.vector.tensor_copy(idx16[:], idxf[:])

            # ---- table stores (contiguous per partition) ----
            st_t = nc.scalar.dma_start(
                out=tcd[0 : 2 * V].rearrange("(p w) d -> p (w d)", p=128),
                in_=tcbf[:],
            )
            st_z = nc.sync.dma_start(out=tcd[2 * V : 2 * V + 1, :], in_=zrow[:])
            tc_stores = [st_t, st_z]

            # ---- chunked gathers (triggered), round-robin over queues ----
            nreg = nc.gpsimd.to_reg(CHUNK)
            for ci in range(NCHUNK):
                q = ci % NQ
                g = gp.tile([128, JB * D], BF16, tag=f"g{ci}")
                gi = nc.gpsimd.dma_gather(
                    g[:].rearrange("p (j d) -> p j d", d=D),
                    tcd[0 : 2 * V + 1],
                    idx16[:, ci * (CHUNK // 16) : (ci + 1) * (CHUNK // 16)],
                    num_idxs=CHUNK,
                    num_idxs_reg=nreg,
                    elem_size=D,
                    single_packet=False,
                    queue_num=q,
                )
                if ci == 0:
                    # RAW guard: gather SDMA reads tcd; Tile does not thread
                    # DRAM deps.  Pool program order covers later chunks.
                    for st in tc_stores:
                        add_dep_helper(gi.ins, st.ins, True, "tcd RAW guard")
                eng = nc.sync if ci % 2 == 0 else nc.scalar
                eng.dma_start(
                    out=out[:, ci * JB * D : (ci + 1) * JB * D], in_=g[:]
                )
    nc.compile()
    return nc


_NC = None


def _pos_tok():
    """token handled by gather position i: (i%16)*512 + i//16."""
    i = np.arange(TPC)
    return (i % 16) * SPC + i // 16


def kernel(x, time_embedding):
    global _NC
    x = np.ascontiguousarray(np.asarray(x, dtype=np.float32))
    t = np.ascontiguousarray(np.asarray(time_embedding, dtype=np.float32))
    xf = x.reshape(-1)
    in_maps = []
    for c in range(N_CORES):
        xc = xf[c * TPC : (c + 1) * TPC].reshape(16, SPC)
        in_maps.append({"xr": np.ascontiguousarray(np.tile(xc, (8, 1))), "emb": t})

    if _NC is None:
        _NC = build()
    res = bass_utils.run_bass_kernel_spmd(_NC, in_maps, core_ids=list(range(N_CORES)))
    global _LAST_RES
    _LAST_RES = res

    tok = _pos_tok()  # position i -> token
    outs = []
    for c in range(N_CORES):
        oc = np.asarray(res.results[c]["out"]).astype(np.float32)
        # oc[p, j*D:(j+1)*D] = gather position i = j*128 + p
        pos = oc.reshape(128, TPC // 128, D).transpose(1, 0, 2).reshape(TPC, D)
        full = np.empty_like(pos)
        full[tok] = pos
        outs.append(full)
    return np.concatenate(outs, axis=0).reshape(B, S, D)
